# revision 34
# baseline (speedup 1.0000x reference)
"""Trainium2 Bass kernel for nn_BaselineModel_55018531061929 (2-layer HSTU-style
dense transformer, B=2 L=2048 D=1024 H=8, SiLU attention).

Sharding (plan D): token-sharded projections + head-sharded attention via
8-core AllToAll. 8 cores = 2 batches x 4 token blocks of 512. Each core:
  - computes Q/K/U/V (all heads, own 512 tokens) locally from fp32 h,
    weights in bf16 (stationary), rope applied locally, outputs bf16;
  - AllToAll #1 reshard: core c receives head-c Q/K/U/V for all 2048 tokens
    of both batches (blocks 0-3 = batch 0, 4-7 = batch 1);
  - attention for head c on 2 batch instances with a STRUCTURAL causal
    triangular loop (query chunk qc only attends key tiles kt <= 4qc+3,
    diagonal 4 tiles masked from data) - perfectly load balanced;
  - AllToAll #2 returns attention outputs token-sharded; Wo/FFN/LN local
    in fp32 with bf16 stationary weights.
"""

import os
import time

import numpy as np

B, L, D, H, NL = 2, 2048, 1024, 8, 2
HD = D // H
EPS = 1e-8
NC = 8
T = 512            # tokens per core
DT = D // 128      # 8 d-tiles
G8 = [[0, 1, 2, 3, 4, 5, 6, 7]]

_CACHE = {}


# --------------------------------------------------------------------------
# device program
# --------------------------------------------------------------------------

def _build_program(sim=False, unroll=1):
    import concourse.bacc as bacc
    import concourse.mybir as mybir
    import concourse.tile as tile
    from concourse.masks import make_identity

    f32 = mybir.dt.float32
    f32r = mybir.dt.float32r
    bf16 = mybir.dt.bfloat16
    AF = mybir.ActivationFunctionType

    nc = bacc.Bacc("TRN2", target_bir_lowering=False, debug=False,
                   num_devices=1 if sim else NC)

    # ---- I/O ----
    x_in = nc.dram_tensor("x_fm", [D, T], f32r, kind="ExternalInput")
    maskd_in = nc.dram_tensor("maskd", [128, 4, T], bf16, kind="ExternalInput")
    cos_in = nc.dram_tensor("cosf", [128, T], f32, kind="ExternalInput")
    sin_in = nc.dram_tensor("sinf", [128, T], f32, kind="ExternalInput")
    psw_in = nc.dram_tensor("pswap", [128, 128], f32r, kind="ExternalInput")
    w_in = nc.dram_tensor("wstack", [7 * NL, 8, 128, DT, 128], bf16,
                          kind="ExternalInput")
    ones_in = nc.dram_tensor("onesf", [128, 128], f32r, kind="ExternalInput")
    b_in = nc.dram_tensor("bstack", [7 * NL, 128, 8], f32, kind="ExternalInput")
    lng_in = nc.dram_tensor("lng", [2 * NL + 1, 128, DT], f32, kind="ExternalInput")
    lnb_in = nc.dram_tensor("lnb", [2 * NL + 1, 128, DT], f32, kind="ExternalInput")
    out_t = nc.dram_tensor("out_fm", [D, T], f32r, kind="ExternalOutput")

    W_Q, W_K, W_U, W_V, W_O, W_1, W_2 = range(7)
    INV_SQRT_HD = float(1.0 / np.sqrt(HD))

    with tile.TileContext(nc) as tc:
        with (
            tc.tile_pool(name="const", bufs=1) as constp,
            tc.tile_pool(name="acts", bufs=1) as acts,
            tc.tile_pool(name="wcol", bufs=6) as wcolp,
            tc.tile_pool(name="tmp", bufs=6) as tmpp,
            tc.tile_pool(name="small", bufs=4) as smallp,
            tc.tile_pool(name="krp", bufs=4) as krp,
            tc.tile_pool(name="att", bufs=1) as attp,
            tc.tile_pool(name="vtp", bufs=2) as vtp,
            tc.tile_pool(name="wtsp", bufs=22) as wtsp,
            tc.tile_pool(name="bcp", bufs=2) as bcp,
            tc.tile_pool(name="psc", bufs=3, space="PSUM") as pscp,
            tc.tile_pool(name="shr", bufs=2, space="PSUM") as shrp,
            tc.tile_pool(name="dram", bufs=1, space="DRAM") as dramp,
        ):
            # ---- constants ----
            ones_sb = constp.tile([128, 128], f32r, name="ones_sb")
            nc.sync.dma_start(ones_sb[:], ones_in[:])
            ones_col = ones_sb[:, 0:1]
            ones_row = ones_sb[0:1, :]
            eps_col = constp.tile([128, 1], f32, name="eps_col")
            nc.vector.memset(eps_col[:], EPS)
            x_sb = constp.tile([128, DT, T], f32r, name="x_sb")
            x_in_t = x_in.ap().rearrange("(dt p) t -> p dt t", p=128)
            for dt in range(DT):
                nc.sync.dma_start(x_sb[:, dt, :], x_in_t[:, dt, :])
            maskd_sb = constp.tile([128, 4, T], bf16, name="maskd_sb")
            nc.sync.dma_start(maskd_sb[:], maskd_in.ap())
            cos_sb = constp.tile([128, T], f32, name="cos_sb")
            nc.sync.dma_start(cos_sb[:], cos_in[:])
            sin_sb = constp.tile([128, T], f32, name="sin_sb")
            nc.sync.dma_start(sin_sb[:], sin_in[:])
            psw_sb = constp.tile([128, 128], f32r, name="psw_sb")
            nc.sync.dma_start(psw_sb[:], psw_in[:])
            bcol_sb = constp.tile([128, 7 * NL, 8], f32, name="bcol_sb")
            nc.sync.dma_start(bcol_sb[:], b_in.ap().rearrange("w p c -> p w c"))
            lng_sb = constp.tile([128, 2 * NL + 1, DT], f32, name="lng_sb")
            nc.sync.dma_start(lng_sb[:], lng_in.ap().rearrange("w p c -> p w c"))
            lnb_sb = constp.tile([128, 2 * NL + 1, DT], f32, name="lnb_sb")
            nc.sync.dma_start(lnb_sb[:], lnb_in.ap().rearrange("w p c -> p w c"))
            identb = constp.tile([128, 128], bf16, name="identb")
            make_identity(nc, identb)

            # ---- collective buffers ----
            a2a1a_in = [dramp.tile([8, 2, 128, T], bf16, name=f"a2a1a_in{l}")
                        for l in range(NL)]
            a2a1a_out = [dramp.tile([8, 2, 128, T], bf16, name=f"a2a1a_out{l}")
                         for l in range(NL)]
            a2a1b_in = [dramp.tile([8, 128, T], bf16, name=f"a2a1b_in{l}")
                        for l in range(NL)]
            a2a1b_out = [dramp.tile([8, 128, T], bf16, name=f"a2a1b_out{l}")
                         for l in range(NL)]
            a2a2_in = [dramp.tile([8, 128, T], bf16, name=f"a2a2_in{l}")
                       for l in range(NL)]
            a2a2_out = [dramp.tile([8, 128, T], bf16, name=f"a2a2_out{l}")
                        for l in range(NL)]
            wu_in = dramp.tile([8, 128, 16], bf16, name="wu_in")
            wu_out = dramp.tile([8, 128, 16], bf16, name="wu_out")

            def a2a(ins, outs):
                if sim:
                    for j in range(8):
                        nc.sync.dma_start(outs[j], ins[j])
                else:
                    nc.gpsimd.collective_compute(
                        "AllToAll", mybir.AluOpType.bypass,
                        replica_groups=G8, ins=[ins[:]], outs=[outs[:]])

            def load_wcol(widx, ot):
                w = wcolp.tile([128, DT, 128], bf16, name="wct", tag="wct")
                nc.sync.dma_start(w[:], w_in[widx, ot])
                return w

            def ln_stats():
                """LN stats over x_sb -> bc[:,0,:]=mean bcast, bc[:,1,:]=istd
                bcast."""
                ps_sum = shrp.tile([1, T], f32, name="ps_sum", tag="shr",
                                   padded_shape=[128, T])
                ps_sq = shrp.tile([1, T], f32, name="ps_sq", tag="shr",
                                  padded_shape=[128, T])
                for dt in range(DT):
                    sqv = tmpp.tile([128, T], f32r, name="sqv", tag="tmp")
                    nc.scalar.square(sqv[:], x_sb[:, dt, :])
                    nc.tensor.matmul(ps_sum[:], ones_col[:], x_sb[:, dt, :],
                                     start=dt == 0, stop=dt == DT - 1)
                    nc.tensor.matmul(ps_sq[:], ones_col[:], sqv[:],
                                     start=dt == 0, stop=dt == DT - 1)
                s_mean = smallp.tile([1, T], f32r, name="s_mean", tag="sm")
                nc.vector.tensor_scalar_mul(s_mean[:], ps_sum[:], 1.0 / D)
                bc = bcp.tile([128, 2, T], f32, name="bc", tag="bc")
                bm_ps = shrp.tile([128, T], f32, name="bm_ps", tag="shr")
                nc.tensor.matmul(bm_ps[:], ones_row[:], s_mean[:],
                                 start=True, stop=True)
                nc.vector.tensor_copy(bc[:, 0, :], bm_ps[:])
                s_var = smallp.tile([1, T], f32, name="s_var", tag="sm")
                nc.vector.tensor_scalar_mul(s_var[:], ps_sq[:], 1.0 / D)
                s_msq = smallp.tile([1, T], f32, name="s_msq", tag="sm")
                nc.vector.tensor_mul(s_msq[:], s_mean[:], s_mean[:])
                nc.vector.tensor_sub(s_var[:], s_var[:], s_msq[:])
                s_std = smallp.tile([1, T], f32, name="s_std", tag="sm")
                nc.scalar.activation(s_std[:], s_var[:], AF.Sqrt, bias=eps_col[:1])
                s_istd = smallp.tile([1, T], f32r, name="s_istd", tag="sm")
                with nc.allow_low_precision(reason="f32r is full-width fp32"):
                    nc.vector.reciprocal(s_istd[:], s_std[:])
                bi_ps = shrp.tile([128, T], f32, name="bi_ps", tag="shr")
                nc.tensor.matmul(bi_ps[:], ones_row[:], s_istd[:],
                                 start=True, stop=True)
                nc.vector.tensor_copy(bc[:, 1, :], bi_ps[:])
                return bc

            def ln_norm():
                """xb = bf16((x - mean)*istd); gamma/beta live in the folded
                weights so projections consume xb with a plain bias ACT."""
                bc = ln_stats()
                xb = acts.tile([128, DT, T], bf16, name="xb", tag="bigA")
                for dt in range(DT):
                    t1 = tmpp.tile([128, T], f32, name="t1", tag="tmp")
                    nc.vector.tensor_sub(t1[:], x_sb[:, dt, :], bc[:, 0, :])
                    nc.vector.tensor_mul(xb[:, dt, :], t1[:], bc[:, 1, :])
                return xb

            def layernorm_full(idx):
                """Classic layernorm of x_sb (final LN only), f32r out."""
                bc = ln_stats()
                h = acts.tile([128, DT, T], f32r, name="hf", tag="bigF")
                for dt in range(DT):
                    t1 = tmpp.tile([128, T], f32, name="t1", tag="tmp")
                    nc.vector.tensor_sub(t1[:], x_sb[:, dt, :], bc[:, 0, :])
                    nc.vector.tensor_mul(t1[:], t1[:], bc[:, 1, :])
                    nc.scalar.activation(h[:, dt, :], t1[:], AF.Identity,
                                         bias=lnb_sb[:, idx, dt:dt + 1],
                                         scale=lng_sb[:, idx, dt:dt + 1])
                return h

            def proj_pair_psum(widx, otp, rhs_tile):
                """[128, 2, T] psum: halves = ot 2*otp, 2*otp+1 accumulation."""
                w0 = load_wcol(widx, 2 * otp)
                w1 = load_wcol(widx, 2 * otp + 1)
                ps = pscp.tile([128, 2, T], f32, name="ps_p", tag="psc")
                for dt in range(DT):
                    nc.tensor.matmul(ps[:, 0, :], w0[:, dt, :],
                                     rhs_tile[:, dt, :],
                                     start=dt == 0, stop=dt == DT - 1)
                    nc.tensor.matmul(ps[:, 1, :], w1[:, dt, :],
                                     rhs_tile[:, dt, :],
                                     start=dt == 0, stop=dt == DT - 1)
                return ps

            def rope_to_bf16(dst_ap, src_tile):
                """dst(bf16) = src*cosf + (pswap@src)*sinf, one rounding."""
                psw = shrp.tile([128, T], f32, name="psw_ps", tag="shr")
                nc.tensor.matmul(psw[:], psw_sb[:], src_tile[:],
                                 start=True, stop=True)
                t1 = tmpp.tile([128, T], f32, name="rt1", tag="tmp")
                nc.vector.tensor_mul(t1[:], src_tile[:], cos_sb[:])
                t2 = tmpp.tile([128, T], f32, name="rt2", tag="tmp")
                nc.vector.tensor_mul(t2[:], psw[:], sin_sb[:])
                nc.vector.tensor_add(dst_ap, t1[:], t2[:])

            # warmup collective: absorbs first-op slowness off the critical path
            wuc = constp.tile([128, 16], bf16, name="wuc")
            nc.vector.tensor_copy(wuc[:], ones_sb[:, 0:16])
            for j in range(8):
                nc.sync.dma_start(wu_in[j], wuc[:])
            a2a(wu_in, wu_out)

            for rep in range(unroll):
                if rep > 0:
                    for dt in range(DT):
                        nc.sync.dma_start(x_sb[:, dt, :], x_in_t[:, dt, :])
                for layer in range(NL):
                    wofs = 7 * layer
                    xb = ln_norm()

                    # ---- Q/K projections -> A2A1a; V -> A2A1b; U local ----
                    for m, widx in ((0, W_Q), (1, W_K), (3, W_V)):
                        for otp in range(H // 2):
                            ps = proj_pair_psum(wofs + widx, otp, xb)
                            for j in range(2):
                                ot = 2 * otp + j
                                kr = krp.tile([128, T], bf16, name="kr",
                                              tag="kr")
                                if m < 2:  # Q, K: bias then rope
                                    qt = tmpp.tile([128, T], f32r, name="qt",
                                                   tag="tmp")
                                    nc.scalar.activation(
                                        qt[:], ps[:, j, :], AF.Identity,
                                        bias=bcol_sb[:, wofs + widx, ot:ot + 1])
                                    rope_to_bf16(kr[:], qt)
                                    nc.sync.dma_start(
                                        a2a1a_in[layer][ot, m], kr[:])
                                else:  # V
                                    nc.scalar.activation(
                                        kr[:], ps[:, j, :], AF.Identity,
                                        bias=bcol_sb[:, wofs + widx, ot:ot + 1])
                                    nc.sync.dma_start(
                                        a2a1b_in[layer][ot], kr[:])
                        if m == 1:
                            a2a(a2a1a_in[layer], a2a1a_out[layer])
                        elif m == 3:
                            a2a(a2a1b_in[layer], a2a1b_out[layer])
                    # U projection stays local (fills the A2A flight time)
                    u_sb = acts.tile([128, H, T], bf16, name="u_sb", tag="u")
                    for otp in range(H // 2):
                        ps = proj_pair_psum(wofs + W_U, otp, xb)
                        for j in range(2):
                            ot = 2 * otp + j
                            nc.scalar.activation(
                                u_sb[:, ot, :], ps[:, j, :], AF.Identity,
                                bias=bcol_sb[:, wofs + W_U, ot:ot + 1])

                    # ---- attention: head `core`, 2 batch instances ----
                    for inst in range(2):
                        base = 4 * inst
                        qf = attp.tile([128, 4, T], bf16, name="qf", tag="qf")
                        kf = attp.tile([128, 4, T], bf16, name="kf", tag="kf")
                        vf = attp.tile([128, 4, T], bf16, name="vf", tag="vf")
                        for blk in range(4):
                            nc.sync.dma_start(qf[:, blk, :],
                                              a2a1a_out[layer][base + blk, 0])
                            nc.sync.dma_start(kf[:, blk, :],
                                              a2a1a_out[layer][base + blk, 1])
                            nc.sync.dma_start(vf[:, blk, :],
                                              a2a1b_out[layer][base + blk])
                        kflat = kf[:].rearrange("p b t -> p (b t)")
                        # scores phase: all (qc, ktp) pairs -> wt tiles
                        wts = {}
                        for qc in range(4):
                            for ktp in range(2 * qc + 2):
                                psc = pscp.tile([128, 2, T], f32, name="psc",
                                                tag="psc")
                                for j in range(2):
                                    kt = 2 * ktp + j
                                    nc.tensor.matmul(
                                        psc[:, j, :],
                                        kflat[:, kt * 128:(kt + 1) * 128],
                                        qf[:, qc, :], start=True, stop=True)
                                wt = wtsp.tile([128, 2, T], bf16, name="wt",
                                               tag="wt")
                                nc.scalar.activation(wt[:], psc[:], AF.Silu,
                                                     scale=INV_SQRT_HD)
                                if ktp >= 2 * qc:  # diagonal pair: mask
                                    dj = 2 * (ktp - 2 * qc)
                                    nc.vector.tensor_mul(
                                        wt[:], wt[:],
                                        maskd_sb[:, dj:dj + 2, :])
                                wts[(qc, ktp)] = wt
                        # V transposes: vt[kt] = V[128 keys, 128 hd]
                        vt = vtp.tile([128, 16, 128], bf16, name="vt", tag="vt")
                        vflat = vf[:].rearrange("p b t -> p (b t)")
                        for kt in range(16):
                            pst = shrp.tile([128, 128], bf16, name="pst",
                                            tag="shr", padded_shape=[128, 512])
                            nc.tensor.transpose(
                                pst[:], vflat[:, kt * 128:(kt + 1) * 128],
                                identb[:])
                            nc.vector.tensor_copy(vt[:, kt, :], pst[:])
                        # AV phase
                        for qc in range(4):
                            npair = 2 * qc + 2
                            pav = shrp.tile([128, T], f32, name="pav", tag="shr")
                            for ktp in range(npair):
                                wt = wts[(qc, ktp)]
                                for j in range(2):
                                    kt = 2 * ktp + j
                                    nc.tensor.matmul(
                                        pav[:], vt[:, kt, :], wt[:, j, :],
                                        start=kt == 0,
                                        stop=kt == 2 * npair - 1)
                            ao = krp.tile([128, T], bf16, name="ao", tag="kr")
                            nc.scalar.activation(ao[:], pav[:], AF.Identity)
                            nc.sync.dma_start(a2a2_in[layer][base + qc], ao[:])
                    a2a(a2a2_in[layer], a2a2_out[layer])

                    # ---- U gating + output projection + residual ----
                    aa = attp.tile([128, 8, T], bf16, name="aa", tag="aa")
                    for s in range(8):
                        nc.sync.dma_start(aa[:, s, :], a2a2_out[layer][s])
                    au = attp.tile([128, 8, T], bf16, name="au", tag="au")
                    for s in range(8):
                        nc.vector.tensor_mul(au[:, s, :], aa[:, s, :],
                                             u_sb[:, s, :])
                    for otp in range(DT // 2):
                        ps = proj_pair_psum(wofs + W_O, otp, au)
                        for j in range(2):
                            ot = 2 * otp + j
                            otmp = tmpp.tile([128, T], f32, name="otmp",
                                             tag="tmp")
                            nc.vector.tensor_scalar_add(
                                otmp[:], ps[:, j, :],
                                bcol_sb[:, wofs + W_O, ot:ot + 1])
                            nc.vector.tensor_add(x_sb[:, ot, :],
                                                 x_sb[:, ot, :], otmp[:])

                    # ---- FFN (LN2 gain/shift folded into W1) ----
                    xb2 = ln_norm()
                    p_sb = acts.tile([128, DT, T], f32, name="p_sb", tag="p")
                    for otp in range(DT // 2):
                        ps = proj_pair_psum(wofs + W_1, otp, xb2)
                        for j in range(2):
                            ot = 2 * otp + j
                            nc.scalar.activation(
                                p_sb[:, ot, :], ps[:, j, :], AF.Identity,
                                bias=bcol_sb[:, wofs + W_1, ot:ot + 1])
                    gp = acts.tile([128, DT, T], bf16, name="gp", tag="bigA")
                    for ot in range(DT):
                        sp = tmpp.tile([128, T], f32, name="sp", tag="tmp")
                        nc.scalar.activation(sp[:], p_sb[:, ot, :], AF.Silu)
                        nc.vector.tensor_mul(gp[:, ot, :], p_sb[:, ot, :], sp[:])
                    for otp in range(DT // 2):
                        ps = proj_pair_psum(wofs + W_2, otp, gp)
                        for j in range(2):
                            ot = 2 * otp + j
                            ftmp = tmpp.tile([128, T], f32, name="ftmp",
                                             tag="tmp")
                            nc.vector.tensor_scalar_add(
                                ftmp[:], ps[:, j, :],
                                bcol_sb[:, wofs + W_2, ot:ot + 1])
                            nc.vector.tensor_add(x_sb[:, ot, :],
                                                 x_sb[:, ot, :], ftmp[:])

                # ---- final layernorm + output ----
                hf = layernorm_full(2 * NL)
                out_t_t = out_t.ap().rearrange("(dt p) t -> p dt t", p=128)
                for dt in range(DT):
                    nc.sync.dma_start(out_t_t[:, dt, :], hf[:, dt, :])

    nc.compile()
    return nc


# --------------------------------------------------------------------------
# host-side preparation
# --------------------------------------------------------------------------

def _host_prep(inputs):
    import ml_dtypes
    bf16 = ml_dtypes.bfloat16

    seqs = np.asarray(inputs["seqs"], np.float32)
    mask = np.asarray(inputs["attn_mask"])

    perm128 = np.concatenate([np.arange(0, 128, 2), np.arange(1, 128, 2)])
    perm_full = np.concatenate([h * 128 + perm128 for h in range(H)])

    def wprep(W):
        WT = np.ascontiguousarray(W.T)
        return np.ascontiguousarray(
            WT.reshape(DT, 128, 8, 128).transpose(2, 1, 0, 3))

    def bcolv(b):
        return np.ascontiguousarray(b.reshape(8, 128).T)

    def lncol(v):
        return np.ascontiguousarray(v.reshape(DT, 128).T)

    wstack, bstack = [], []
    for i in range(NL):
        ln1g = np.asarray(inputs["ln1_g"][i], np.float32)
        ln1b = np.asarray(inputs["ln1_b"][i], np.float32)
        ln2g = np.asarray(inputs["ln2_g"][i], np.float32)
        ln2b = np.asarray(inputs["ln2_b"][i], np.float32)
        for nm in ["Wq", "Wk", "Wu", "Wv", "Wo", "W1", "W2"]:
            Wm = np.asarray(inputs[nm][i], np.float32)
            bm = np.asarray(inputs["b" + nm[1:].lower()][i], np.float32)
            # fold the feeding layernorm's gain/shift into W and b:
            # W @ (g*xn + s) + b = (W*g) @ xn + (b + W @ s)
            if nm in ("Wq", "Wk", "Wu", "Wv"):
                bm = bm + Wm @ ln1b
                Wm = Wm * ln1g[None, :]
            elif nm == "W1":
                bm = bm + Wm @ ln2b
                Wm = Wm * ln2g[None, :]
            if nm in ("Wq", "Wk"):
                Wm = Wm[perm_full]
                bm = bm[perm_full]
            wstack.append(wprep(Wm))
            bstack.append(bcolv(bm))
    wstack = np.ascontiguousarray(np.stack(wstack)).astype(bf16)
    bstack = np.ascontiguousarray(np.stack(bstack), dtype=np.float32)

    lng = np.stack([lncol(np.asarray(inputs["ln1_g"][0], np.float32)),
                    lncol(np.asarray(inputs["ln2_g"][0], np.float32)),
                    lncol(np.asarray(inputs["ln1_g"][1], np.float32)),
                    lncol(np.asarray(inputs["ln2_g"][1], np.float32)),
                    lncol(np.asarray(inputs["lnf_g"], np.float32))])
    lnb = np.stack([lncol(np.asarray(inputs["ln1_b"][0], np.float32)),
                    lncol(np.asarray(inputs["ln2_b"][0], np.float32)),
                    lncol(np.asarray(inputs["ln1_b"][1], np.float32)),
                    lncol(np.asarray(inputs["ln2_b"][1], np.float32)),
                    lncol(np.asarray(inputs["lnf_b"], np.float32))])
    lng = np.ascontiguousarray(lng, dtype=np.float32)
    lnb = np.ascontiguousarray(lnb, dtype=np.float32)

    pos = np.arange(L, dtype=np.float32)
    ar = np.arange(0, HD, 2).astype(np.float32) / np.float32(HD)
    freqs = np.float32(1.0) / np.power(np.float32(10000.0), ar, dtype=np.float32)
    ang = pos[:, None] * freqs[None, :]
    sin_full, cos_full = np.sin(ang).astype(np.float32), np.cos(ang).astype(np.float32)

    pswap = np.zeros((128, 128), np.float32)
    for i in range(64):
        pswap[i, 64 + i] = 1.0
        pswap[64 + i, i] = 1.0

    in_maps = []
    for c in range(NC):
        b_idx, q0 = c // 4, (c % 4) * T
        cos_t = cos_full[q0:q0 + T].T
        sin_t = sin_full[q0:q0 + T].T
        # diagonal-relative causal mask: maskd[k, j, q] = allow(query q,
        # key 128j+k) within a 512x512 diagonal block
        mdiag = mask[b_idx, 0:T, 0:T]          # [q, k] bool (tril)
        maskd = np.ascontiguousarray(
            mdiag.T.reshape(4, 128, T).transpose(1, 0, 2)
        ).astype(np.float32).astype(bf16)
        in_maps.append({
            "x_fm": np.ascontiguousarray(seqs[b_idx, q0:q0 + T].T),
            "maskd": maskd,
            "cosf": np.ascontiguousarray(np.concatenate([cos_t, cos_t], 0)),
            "sinf": np.ascontiguousarray(np.concatenate([-sin_t, sin_t], 0)),
            "pswap": pswap, "onesf": np.ones((128, 128), np.float32),
            "wstack": wstack, "bstack": bstack, "lng": lng, "lnb": lnb,
        })
    return in_maps


def _get_program(unroll=1):
    key = ("nc", unroll)
    if key not in _CACHE:
        os.environ.setdefault("JAX_PLATFORMS", "")
        _CACHE[key] = _build_program(unroll=unroll)
    return _CACHE[key]


class _Runner:
    """Compile-once jitted SPMD runner over the axon/PJRT path."""

    def __init__(self, nc):
        import jax
        from jax.experimental.shard_map import shard_map
        from jax.sharding import Mesh, PartitionSpec, NamedSharding
        import concourse.bass2jax as bass2jax
        import concourse.mybir as mybir

        self.jax = jax
        self.nc = nc
        bass2jax.install_neuronx_cc_hook()
        partition_name = (nc.partition_id_tensor.name
                          if nc.partition_id_tensor else None)
        in_names, out_names, out_avals, zero_outs = [], [], [], []
        for alloc in nc.m.functions[0].allocations:
            if not isinstance(alloc, mybir.MemoryLocationSet):
                continue
            name = alloc.memorylocations[0].name
            if alloc.kind == "ExternalInput":
                if name != partition_name:
                    in_names.append(name)
            elif alloc.kind == "ExternalOutput":
                out_names.append(name)
                shape = tuple(alloc.tensor_shape)
                dtype = mybir.dt.np(alloc.dtype)
                out_avals.append(jax.core.ShapedArray(shape, dtype))
                zero_outs.append(np.zeros(shape, dtype))
        self.in_names, self.out_names = in_names, out_names
        self.zero_outs = zero_outs
        n_params = len(in_names)
        all_names = in_names + out_names + (
            [partition_name] if partition_name else [])

        def _body(*args):
            operands = list(args)
            if partition_name is not None:
                operands.append(bass2jax.partition_id_tensor())
            return tuple(bass2jax._bass_exec_p.bind(
                *operands, out_avals=tuple(out_avals),
                in_names=tuple(all_names), out_names=tuple(out_names),
                lowering_input_output_aliases=(),
                sim_require_finite=True, sim_require_nnan=True, nc=nc))

        mesh = Mesh(np.asarray(jax.devices()[:NC]), ("core",))
        n_outs = len(out_names)
        self.fn = jax.jit(
            shard_map(_body, mesh=mesh,
                      in_specs=(PartitionSpec("core"),) * (n_params + n_outs),
                      out_specs=(PartitionSpec("core"),) * n_outs,
                      check_rep=False),
            donate_argnums=tuple(range(n_params, n_params + n_outs)),
            keep_unused=True)
        self.shard = NamedSharding(mesh, PartitionSpec("core"))

    def put_inputs(self, in_maps):
        jax = self.jax
        concat_in = [
            np.concatenate([np.asarray(in_maps[c][nm])[None]
                            for c in range(NC)], axis=0)
            .reshape(NC * in_maps[0][nm].shape[0], *in_maps[0][nm].shape[1:])
            for nm in self.in_names]
        return [jax.device_put(a, self.shard) for a in concat_in]

    def fresh_zeros(self):
        jax = self.jax
        return [jax.device_put(
            np.zeros((NC * z.shape[0], *z.shape[1:]), z.dtype), self.shard)
            for z in self.zero_outs]

    def run(self, in_arrs):
        outs = self.fn(*in_arrs, *self.fresh_zeros())
        self.jax.block_until_ready(outs)
        return outs

    def times(self, in_arrs, iters):
        jax = self.jax
        self.run(in_arrs)  # warmup
        ts = []
        for _ in range(iters):
            zs = self.fresh_zeros()
            jax.block_until_ready(zs)
            t0 = time.perf_counter()
            outs = self.fn(*in_arrs, *zs)
            jax.block_until_ready(outs)
            ts.append(time.perf_counter() - t0)
        return ts


def _out_to_full(runner, outs):
    out = np.zeros((B, L, D), np.float32)
    arr0 = np.asarray(outs[runner.out_names.index("out_fm")]).reshape(NC, D, T)
    for c in range(NC):
        b_idx, q0 = c // 4, (c % 4) * T
        out[b_idx, q0:q0 + T] = arr0[c].T
    return out


def kernel(**inputs):
    from concourse.bass_utils import run_bass_kernel_spmd
    in_maps = _host_prep(inputs)
    nc = _get_program(unroll=1)
    res = run_bass_kernel_spmd(nc, in_maps, core_ids=list(range(NC)))
    out = np.zeros((B, L, D), np.float32)
    for c in range(NC):
        b_idx, q0 = c // 4, (c % 4) * T
        out[b_idx, q0:q0 + T] = res.results[c]["out_fm"].T
    return out


def bench_hw(inputs, unroll=6, iters=8):
    """Correctness output + device-time estimate via unrolled NEFF diff."""
    in_maps = _host_prep(inputs)
    r1 = _CACHE.setdefault("runner1", _Runner(_get_program(unroll=1)))
    rN = _CACHE.setdefault(f"runner{unroll}",
                           _Runner(_get_program(unroll=unroll)))
    in1 = r1.put_inputs(in_maps)
    inN = rN.put_inputs(in_maps)
    outs = r1.run(in1)
    full = _out_to_full(r1, outs)
    # interleave timing rounds so dispatch-floor drift cancels
    r1.run(in1)
    rN.run(inN)
    t1s, tNs = [], []
    for _ in range(iters):
        t1s.extend(r1.times(in1, 1))
        tNs.extend(rN.times(inN, 1))
    t1, tN = min(t1s), min(tNs)
    est = (tN - t1) / (unroll - 1)
    # also check unrolled output matches (reps are idempotent)
    fullN = _out_to_full(rN, rN.run(inN))
    assert np.allclose(full, fullN, atol=1e-5), "unrolled output mismatch"
    return full, est, {"t1": t1, "tN": tN, "unroll": unroll,
                       "t1s": t1s, "tNs": tNs}


# revision 35
# speedup vs baseline: 1.1005x; 1.1005x over previous
"""Trainium2 Bass kernel for nn_BaselineModel_55018531061929 (2-layer HSTU-style
dense transformer, B=2 L=2048 D=1024 H=8, SiLU attention).

Sharding (plan D): token-sharded projections + head-sharded attention via
8-core AllToAll. 8 cores = 2 batches x 4 token blocks of 512. Each core:
  - computes Q/K/U/V (all heads, own 512 tokens) locally from fp32 h,
    weights in bf16 (stationary), rope applied locally, outputs bf16;
  - AllToAll #1 reshard: core c receives head-c Q/K/U/V for all 2048 tokens
    of both batches (blocks 0-3 = batch 0, 4-7 = batch 1);
  - attention for head c on 2 batch instances with a STRUCTURAL causal
    triangular loop (query chunk qc only attends key tiles kt <= 4qc+3,
    diagonal 4 tiles masked from data) - perfectly load balanced;
  - AllToAll #2 returns attention outputs token-sharded; Wo/FFN/LN local
    in fp32 with bf16 stationary weights.
"""

import os
import time

import numpy as np

B, L, D, H, NL = 2, 2048, 1024, 8, 2
HD = D // H
EPS = 1e-8
NC = 8
T = 512            # tokens per core
DT = D // 128      # 8 d-tiles
G8 = [[0, 1, 2, 3, 4, 5, 6, 7]]

_CACHE = {}


# --------------------------------------------------------------------------
# device program
# --------------------------------------------------------------------------

def _build_program(sim=False, unroll=1):
    import concourse.bacc as bacc
    import concourse.mybir as mybir
    import concourse.tile as tile
    from concourse.masks import make_identity

    f32 = mybir.dt.float32
    f32r = mybir.dt.float32r
    bf16 = mybir.dt.bfloat16
    AF = mybir.ActivationFunctionType

    nc = bacc.Bacc("TRN2", target_bir_lowering=False, debug=False,
                   num_devices=1 if sim else NC)

    # ---- I/O ----
    x_in = nc.dram_tensor("x_fm", [D, T], f32r, kind="ExternalInput")
    maskd_in = nc.dram_tensor("maskd", [128, 4, T], bf16, kind="ExternalInput")
    cos_in = nc.dram_tensor("cosf", [128, T], f32, kind="ExternalInput")
    sin_in = nc.dram_tensor("sinf", [128, T], f32, kind="ExternalInput")
    psw_in = nc.dram_tensor("pswap", [128, 128], f32r, kind="ExternalInput")
    w_in = nc.dram_tensor("wstack", [7 * NL, 8, 128, DT, 128], bf16,
                          kind="ExternalInput")
    ones_in = nc.dram_tensor("onesf", [128, 128], f32r, kind="ExternalInput")
    b_in = nc.dram_tensor("bstack", [7 * NL, 128, 8], f32, kind="ExternalInput")
    lng_in = nc.dram_tensor("lng", [2 * NL + 1, 128, DT], f32, kind="ExternalInput")
    lnb_in = nc.dram_tensor("lnb", [2 * NL + 1, 128, DT], f32, kind="ExternalInput")
    out_t = nc.dram_tensor("out_fm", [D, T], f32r, kind="ExternalOutput")

    W_Q, W_K, W_U, W_V, W_O, W_1, W_2 = range(7)
    INV_SQRT_HD = float(1.0 / np.sqrt(HD))

    with tile.TileContext(nc) as tc:
        with (
            tc.tile_pool(name="const", bufs=1) as constp,
            tc.tile_pool(name="acts", bufs=1) as acts,
            tc.tile_pool(name="wcol", bufs=10) as wcolp,
            tc.tile_pool(name="tmp", bufs=6) as tmpp,
            tc.tile_pool(name="small", bufs=4) as smallp,
            tc.tile_pool(name="krp", bufs=4) as krp,
            tc.tile_pool(name="att", bufs=1) as attp,
            tc.tile_pool(name="vtp", bufs=2) as vtp,
            tc.tile_pool(name="wtsp", bufs=20) as wtsp,
            tc.tile_pool(name="bcp", bufs=2) as bcp,
            tc.tile_pool(name="psc", bufs=3, space="PSUM") as pscp,
            tc.tile_pool(name="shr", bufs=2, space="PSUM") as shrp,
            tc.tile_pool(name="dram", bufs=1, space="DRAM") as dramp,
        ):
            # ---- constants ----
            ones_sb = constp.tile([128, 128], f32r, name="ones_sb")
            nc.sync.dma_start(ones_sb[:], ones_in[:])
            ones_col = ones_sb[:, 0:1]
            ones_f = constp.tile([1, 128], f32, name="ones_f")
            nc.vector.memset(ones_f[:], 1.0)
            ones_row = ones_f[0:1, :]
            eps_col = constp.tile([128, 1], f32, name="eps_col")
            nc.vector.memset(eps_col[:], EPS)
            x_sb = constp.tile([128, DT, T], f32r, name="x_sb")
            x_in_t = x_in.ap().rearrange("(dt p) t -> p dt t", p=128)
            for dt in range(DT):
                nc.sync.dma_start(x_sb[:, dt, :], x_in_t[:, dt, :])
            maskd_sb = constp.tile([128, 4, T], bf16, name="maskd_sb")
            nc.sync.dma_start(maskd_sb[:], maskd_in.ap())
            cos_sb = constp.tile([128, T], f32, name="cos_sb")
            nc.sync.dma_start(cos_sb[:], cos_in[:])
            sin_sb = constp.tile([128, T], f32, name="sin_sb")
            nc.sync.dma_start(sin_sb[:], sin_in[:])
            psw_sb = constp.tile([128, 128], f32r, name="psw_sb")
            nc.sync.dma_start(psw_sb[:], psw_in[:])
            bcol_sb = constp.tile([128, 7 * NL, 8], f32, name="bcol_sb")
            nc.sync.dma_start(bcol_sb[:], b_in.ap().rearrange("w p c -> p w c"))
            lng_sb = constp.tile([128, 2 * NL + 1, DT], f32, name="lng_sb")
            nc.sync.dma_start(lng_sb[:], lng_in.ap().rearrange("w p c -> p w c"))
            lnb_sb = constp.tile([128, 2 * NL + 1, DT], f32, name="lnb_sb")
            nc.sync.dma_start(lnb_sb[:], lnb_in.ap().rearrange("w p c -> p w c"))
            identb = constp.tile([128, 128], bf16, name="identb")
            make_identity(nc, identb)

            # ---- collective buffers ----
            a2a1a_in = [dramp.tile([8, 2, 128, T], bf16, name=f"a2a1a_in{l}")
                        for l in range(NL)]
            a2a1a_out = [dramp.tile([8, 2, 128, T], bf16, name=f"a2a1a_out{l}")
                         for l in range(NL)]
            a2a1b_in = [dramp.tile([8, 128, T], bf16, name=f"a2a1b_in{l}")
                        for l in range(NL)]
            a2a1b_out = [dramp.tile([8, 128, T], bf16, name=f"a2a1b_out{l}")
                         for l in range(NL)]
            a2a2_in = [dramp.tile([8, 128, T], bf16, name=f"a2a2_in{l}")
                       for l in range(NL)]
            a2a2_out = [dramp.tile([8, 128, T], bf16, name=f"a2a2_out{l}")
                        for l in range(NL)]
            wu_in = dramp.tile([8, 128, 16], bf16, name="wu_in")
            wu_out = dramp.tile([8, 128, 16], bf16, name="wu_out")

            def a2a(ins, outs):
                if sim:
                    for j in range(8):
                        nc.sync.dma_start(outs[j], ins[j])
                else:
                    nc.gpsimd.collective_compute(
                        "AllToAll", mybir.AluOpType.bypass,
                        replica_groups=G8, ins=[ins[:]], outs=[outs[:]])

            def load_wcol(widx, ot):
                w = wcolp.tile([128, DT, 128], bf16, name="wct", tag="wct")
                nc.sync.dma_start(w[:], w_in[widx, ot])
                return w

            def ln_stats():
                """LN stats over x_sb -> bc[:,0,:]=mean bcast, bc[:,1,:]=istd
                bcast."""
                ps_sum = shrp.tile([1, T], f32, name="ps_sum", tag="shr",
                                   padded_shape=[128, T])
                ps_sq = shrp.tile([1, T], f32, name="ps_sq", tag="shr",
                                  padded_shape=[128, T])
                for dt in range(DT):
                    sqv = tmpp.tile([128, T], f32r, name="sqv", tag="tmp")
                    nc.scalar.square(sqv[:], x_sb[:, dt, :])
                    nc.tensor.matmul(ps_sum[:], ones_col[:], x_sb[:, dt, :],
                                     start=dt == 0, stop=dt == DT - 1)
                    nc.tensor.matmul(ps_sq[:], ones_col[:], sqv[:],
                                     start=dt == 0, stop=dt == DT - 1)
                s_mean = smallp.tile([1, T], f32, name="s_mean", tag="sm")
                nc.vector.tensor_scalar_mul(s_mean[:], ps_sum[:], 1.0 / D)
                bc = bcp.tile([128, 2, T], f32, name="bc", tag="bc")
                bm_ps = shrp.tile([128, T], f32, name="bm_ps", tag="shr")
                nc.tensor.matmul(bm_ps[:], ones_row[:], s_mean[:],
                                 start=True, stop=True)
                nc.vector.tensor_copy(bc[:, 0, :], bm_ps[:])
                s_var = smallp.tile([1, T], f32, name="s_var", tag="sm")
                nc.vector.tensor_scalar_mul(s_var[:], ps_sq[:], 1.0 / D)
                s_msq = smallp.tile([1, T], f32, name="s_msq", tag="sm")
                nc.vector.tensor_mul(s_msq[:], s_mean[:], s_mean[:])
                nc.vector.tensor_sub(s_var[:], s_var[:], s_msq[:])
                s_std = smallp.tile([1, T], f32, name="s_std", tag="sm")
                nc.scalar.activation(s_std[:], s_var[:], AF.Sqrt, bias=eps_col[:1])
                s_istd = smallp.tile([1, T], f32, name="s_istd", tag="sm")
                nc.vector.reciprocal_approx_fast(s_istd[:], s_std[:])
                bi_ps = shrp.tile([128, T], f32, name="bi_ps", tag="shr")
                nc.tensor.matmul(bi_ps[:], ones_row[:], s_istd[:],
                                 start=True, stop=True)
                nc.vector.tensor_copy(bc[:, 1, :], bi_ps[:])
                return bc

            def ln_norm():
                """xb = bf16((x - mean)*istd); gamma/beta live in the folded
                weights so projections consume xb with a plain bias ACT."""
                bc = ln_stats()
                xb = acts.tile([128, DT, T], bf16, name="xb", tag="bigA")
                for dt in range(DT):
                    t1 = tmpp.tile([128, T], f32, name="t1", tag="tmp")
                    nc.vector.tensor_sub(t1[:], x_sb[:, dt, :], bc[:, 0, :])
                    nc.vector.tensor_mul(xb[:, dt, :], t1[:], bc[:, 1, :])
                return xb

            def layernorm_full(idx):
                """Classic layernorm of x_sb (final LN only), f32r out."""
                bc = ln_stats()
                h = acts.tile([128, DT, T], f32r, name="hf", tag="bigF")
                for dt in range(DT):
                    t1 = tmpp.tile([128, T], f32, name="t1", tag="tmp")
                    nc.vector.tensor_sub(t1[:], x_sb[:, dt, :], bc[:, 0, :])
                    nc.vector.tensor_mul(t1[:], t1[:], bc[:, 1, :])
                    nc.scalar.activation(h[:, dt, :], t1[:], AF.Identity,
                                         bias=lnb_sb[:, idx, dt:dt + 1],
                                         scale=lng_sb[:, idx, dt:dt + 1])
                return h

            def proj_pair_psum(widx, otp, rhs_tile):
                """[128, 2, T] psum: halves = ot 2*otp, 2*otp+1 accumulation."""
                w0 = load_wcol(widx, 2 * otp)
                w1 = load_wcol(widx, 2 * otp + 1)
                ps = pscp.tile([128, 2, T], f32, name="ps_p", tag="psc")
                for dt in range(DT):
                    nc.tensor.matmul(ps[:, 0, :], w0[:, dt, :],
                                     rhs_tile[:, dt, :],
                                     start=dt == 0, stop=dt == DT - 1)
                    nc.tensor.matmul(ps[:, 1, :], w1[:, dt, :],
                                     rhs_tile[:, dt, :],
                                     start=dt == 0, stop=dt == DT - 1)
                return ps

            def rope_to_bf16(dst_ap, src_tile):
                """dst(bf16) = src*cosf + (pswap@src)*sinf, one rounding."""
                psw = shrp.tile([128, T], f32, name="psw_ps", tag="shr")
                nc.tensor.matmul(psw[:], psw_sb[:], src_tile[:],
                                 start=True, stop=True)
                t1 = tmpp.tile([128, T], f32, name="rt1", tag="tmp")
                nc.vector.tensor_mul(t1[:], src_tile[:], cos_sb[:])
                t2 = tmpp.tile([128, T], f32, name="rt2", tag="tmp")
                nc.vector.tensor_mul(t2[:], psw[:], sin_sb[:])
                nc.vector.tensor_add(dst_ap, t1[:], t2[:])

            # warmup collective: absorbs first-op slowness off the critical path
            wuc = constp.tile([128, 16], bf16, name="wuc")
            nc.vector.tensor_copy(wuc[:], ones_sb[:, 0:16])
            for j in range(8):
                nc.sync.dma_start(wu_in[j], wuc[:])
            a2a(wu_in, wu_out)

            for rep in range(unroll):
                if rep > 0:
                    for dt in range(DT):
                        nc.sync.dma_start(x_sb[:, dt, :], x_in_t[:, dt, :])
                for layer in range(NL):
                    wofs = 7 * layer
                    xb = ln_norm()

                    # ---- Q/K projections -> A2A1a; V -> A2A1b; U local ----
                    for m, widx in ((0, W_Q), (1, W_K), (3, W_V)):
                        for otp in range(H // 2):
                            ps = proj_pair_psum(wofs + widx, otp, xb)
                            for j in range(2):
                                ot = 2 * otp + j
                                kr = krp.tile([128, T], bf16, name="kr",
                                              tag="kr")
                                if m < 2:  # Q, K: bias then rope
                                    qt = tmpp.tile([128, T], f32r, name="qt",
                                                   tag="tmp")
                                    nc.scalar.activation(
                                        qt[:], ps[:, j, :], AF.Identity,
                                        bias=bcol_sb[:, wofs + widx, ot:ot + 1])
                                    rope_to_bf16(kr[:], qt)
                                    nc.sync.dma_start(
                                        a2a1a_in[layer][ot, m], kr[:])
                                else:  # V
                                    nc.scalar.activation(
                                        kr[:], ps[:, j, :], AF.Identity,
                                        bias=bcol_sb[:, wofs + widx, ot:ot + 1])
                                    nc.sync.dma_start(
                                        a2a1b_in[layer][ot], kr[:])
                        if m == 1:
                            a2a(a2a1a_in[layer], a2a1a_out[layer])
                        elif m == 3:
                            a2a(a2a1b_in[layer], a2a1b_out[layer])
                    # U projection stays local (fills the A2A flight time)
                    u_sb = acts.tile([128, H, T], bf16, name="u_sb", tag="u")
                    for otp in range(H // 2):
                        ps = proj_pair_psum(wofs + W_U, otp, xb)
                        for j in range(2):
                            ot = 2 * otp + j
                            nc.scalar.activation(
                                u_sb[:, ot, :], ps[:, j, :], AF.Identity,
                                bias=bcol_sb[:, wofs + W_U, ot:ot + 1])

                    # ---- attention: head `core`, 2 batch instances ----
                    for inst in range(2):
                        base = 4 * inst
                        qf = attp.tile([128, 4, T], bf16, name="qf", tag="qf")
                        kf = attp.tile([128, 4, T], bf16, name="kf", tag="kf")
                        vf = attp.tile([128, 4, T], bf16, name="vf", tag="vf")
                        for blk in range(4):
                            nc.sync.dma_start(qf[:, blk, :],
                                              a2a1a_out[layer][base + blk, 0])
                            nc.sync.dma_start(kf[:, blk, :],
                                              a2a1a_out[layer][base + blk, 1])
                            nc.sync.dma_start(vf[:, blk, :],
                                              a2a1b_out[layer][base + blk])
                        kflat = kf[:].rearrange("p b t -> p (b t)")
                        # scores phase: all (qc, ktp) pairs -> wt tiles
                        wts = {}
                        for qc in range(4):
                            for ktp in range(2 * qc + 2):
                                psc = pscp.tile([128, 2, T], f32, name="psc",
                                                tag="psc")
                                for j in range(2):
                                    kt = 2 * ktp + j
                                    nc.tensor.matmul(
                                        psc[:, j, :],
                                        kflat[:, kt * 128:(kt + 1) * 128],
                                        qf[:, qc, :], start=True, stop=True)
                                wt = wtsp.tile([128, 2, T], bf16, name="wt",
                                               tag="wt")
                                nc.scalar.activation(wt[:], psc[:], AF.Silu,
                                                     scale=INV_SQRT_HD)
                                if ktp >= 2 * qc:  # diagonal pair: mask
                                    dj = 2 * (ktp - 2 * qc)
                                    nc.vector.tensor_mul(
                                        wt[:], wt[:],
                                        maskd_sb[:, dj:dj + 2, :])
                                wts[(qc, ktp)] = wt
                        # V transposes: vt[kt] = V[128 keys, 128 hd]
                        vt = vtp.tile([128, 16, 128], bf16, name="vt", tag="vt")
                        vflat = vf[:].rearrange("p b t -> p (b t)")
                        for kt in range(16):
                            pst = shrp.tile([128, 128], bf16, name="pst",
                                            tag="shr", padded_shape=[128, 512])
                            nc.tensor.transpose(
                                pst[:], vflat[:, kt * 128:(kt + 1) * 128],
                                identb[:])
                            nc.vector.tensor_copy(vt[:, kt, :], pst[:])
                        # AV phase
                        for qc in range(4):
                            npair = 2 * qc + 2
                            pav = shrp.tile([128, T], f32, name="pav", tag="shr")
                            for ktp in range(npair):
                                wt = wts[(qc, ktp)]
                                for j in range(2):
                                    kt = 2 * ktp + j
                                    nc.tensor.matmul(
                                        pav[:], vt[:, kt, :], wt[:, j, :],
                                        start=kt == 0,
                                        stop=kt == 2 * npair - 1)
                            ao = krp.tile([128, T], bf16, name="ao", tag="kr")
                            nc.scalar.activation(ao[:], pav[:], AF.Identity)
                            nc.sync.dma_start(a2a2_in[layer][base + qc], ao[:])
                    a2a(a2a2_in[layer], a2a2_out[layer])

                    # ---- U gating + output projection + residual ----
                    aa = attp.tile([128, 8, T], bf16, name="aa", tag="aa")
                    for s in range(8):
                        nc.sync.dma_start(aa[:, s, :], a2a2_out[layer][s])
                    au = attp.tile([128, 8, T], bf16, name="au", tag="au")
                    for s in range(8):
                        nc.vector.tensor_mul(au[:, s, :], aa[:, s, :],
                                             u_sb[:, s, :])
                    for otp in range(DT // 2):
                        ps = proj_pair_psum(wofs + W_O, otp, au)
                        for j in range(2):
                            ot = 2 * otp + j
                            otmp = tmpp.tile([128, T], f32, name="otmp",
                                             tag="tmp")
                            nc.vector.tensor_scalar_add(
                                otmp[:], ps[:, j, :],
                                bcol_sb[:, wofs + W_O, ot:ot + 1])
                            nc.vector.tensor_add(x_sb[:, ot, :],
                                                 x_sb[:, ot, :], otmp[:])

                    # ---- FFN (LN2 gain/shift folded into W1) ----
                    xb2 = ln_norm()
                    p_sb = acts.tile([128, DT, T], f32, name="p_sb", tag="p")
                    for otp in range(DT // 2):
                        ps = proj_pair_psum(wofs + W_1, otp, xb2)
                        for j in range(2):
                            ot = 2 * otp + j
                            nc.scalar.activation(
                                p_sb[:, ot, :], ps[:, j, :], AF.Identity,
                                bias=bcol_sb[:, wofs + W_1, ot:ot + 1])
                    gp = acts.tile([128, DT, T], bf16, name="gp", tag="bigA")
                    for ot in range(DT):
                        sp = tmpp.tile([128, T], f32, name="sp", tag="tmp")
                        nc.scalar.activation(sp[:], p_sb[:, ot, :], AF.Silu)
                        nc.vector.tensor_mul(gp[:, ot, :], p_sb[:, ot, :], sp[:])
                    for otp in range(DT // 2):
                        ps = proj_pair_psum(wofs + W_2, otp, gp)
                        for j in range(2):
                            ot = 2 * otp + j
                            ftmp = tmpp.tile([128, T], f32, name="ftmp",
                                             tag="tmp")
                            nc.vector.tensor_scalar_add(
                                ftmp[:], ps[:, j, :],
                                bcol_sb[:, wofs + W_2, ot:ot + 1])
                            nc.vector.tensor_add(x_sb[:, ot, :],
                                                 x_sb[:, ot, :], ftmp[:])

                # ---- final layernorm + output ----
                hf = layernorm_full(2 * NL)
                out_t_t = out_t.ap().rearrange("(dt p) t -> p dt t", p=128)
                for dt in range(DT):
                    nc.sync.dma_start(out_t_t[:, dt, :], hf[:, dt, :])

    nc.compile()
    return nc


# --------------------------------------------------------------------------
# host-side preparation
# --------------------------------------------------------------------------

def _host_prep(inputs):
    import ml_dtypes
    bf16 = ml_dtypes.bfloat16

    seqs = np.asarray(inputs["seqs"], np.float32)
    mask = np.asarray(inputs["attn_mask"])

    perm128 = np.concatenate([np.arange(0, 128, 2), np.arange(1, 128, 2)])
    perm_full = np.concatenate([h * 128 + perm128 for h in range(H)])

    def wprep(W):
        WT = np.ascontiguousarray(W.T)
        return np.ascontiguousarray(
            WT.reshape(DT, 128, 8, 128).transpose(2, 1, 0, 3))

    def bcolv(b):
        return np.ascontiguousarray(b.reshape(8, 128).T)

    def lncol(v):
        return np.ascontiguousarray(v.reshape(DT, 128).T)

    wstack, bstack = [], []
    for i in range(NL):
        ln1g = np.asarray(inputs["ln1_g"][i], np.float32)
        ln1b = np.asarray(inputs["ln1_b"][i], np.float32)
        ln2g = np.asarray(inputs["ln2_g"][i], np.float32)
        ln2b = np.asarray(inputs["ln2_b"][i], np.float32)
        for nm in ["Wq", "Wk", "Wu", "Wv", "Wo", "W1", "W2"]:
            Wm = np.asarray(inputs[nm][i], np.float32)
            bm = np.asarray(inputs["b" + nm[1:].lower()][i], np.float32)
            # fold the feeding layernorm's gain/shift into W and b:
            # W @ (g*xn + s) + b = (W*g) @ xn + (b + W @ s)
            if nm in ("Wq", "Wk", "Wu", "Wv"):
                bm = bm + Wm @ ln1b
                Wm = Wm * ln1g[None, :]
            elif nm == "W1":
                bm = bm + Wm @ ln2b
                Wm = Wm * ln2g[None, :]
            if nm in ("Wq", "Wk"):
                Wm = Wm[perm_full]
                bm = bm[perm_full]
            wstack.append(wprep(Wm))
            bstack.append(bcolv(bm))
    wstack = np.ascontiguousarray(np.stack(wstack)).astype(bf16)
    bstack = np.ascontiguousarray(np.stack(bstack), dtype=np.float32)

    lng = np.stack([lncol(np.asarray(inputs["ln1_g"][0], np.float32)),
                    lncol(np.asarray(inputs["ln2_g"][0], np.float32)),
                    lncol(np.asarray(inputs["ln1_g"][1], np.float32)),
                    lncol(np.asarray(inputs["ln2_g"][1], np.float32)),
                    lncol(np.asarray(inputs["lnf_g"], np.float32))])
    lnb = np.stack([lncol(np.asarray(inputs["ln1_b"][0], np.float32)),
                    lncol(np.asarray(inputs["ln2_b"][0], np.float32)),
                    lncol(np.asarray(inputs["ln1_b"][1], np.float32)),
                    lncol(np.asarray(inputs["ln2_b"][1], np.float32)),
                    lncol(np.asarray(inputs["lnf_b"], np.float32))])
    lng = np.ascontiguousarray(lng, dtype=np.float32)
    lnb = np.ascontiguousarray(lnb, dtype=np.float32)

    pos = np.arange(L, dtype=np.float32)
    ar = np.arange(0, HD, 2).astype(np.float32) / np.float32(HD)
    freqs = np.float32(1.0) / np.power(np.float32(10000.0), ar, dtype=np.float32)
    ang = pos[:, None] * freqs[None, :]
    sin_full, cos_full = np.sin(ang).astype(np.float32), np.cos(ang).astype(np.float32)

    pswap = np.zeros((128, 128), np.float32)
    for i in range(64):
        pswap[i, 64 + i] = 1.0
        pswap[64 + i, i] = 1.0

    in_maps = []
    for c in range(NC):
        b_idx, q0 = c // 4, (c % 4) * T
        cos_t = cos_full[q0:q0 + T].T
        sin_t = sin_full[q0:q0 + T].T
        # diagonal-relative causal mask: maskd[k, j, q] = allow(query q,
        # key 128j+k) within a 512x512 diagonal block
        mdiag = mask[b_idx, 0:T, 0:T]          # [q, k] bool (tril)
        maskd = np.ascontiguousarray(
            mdiag.T.reshape(4, 128, T).transpose(1, 0, 2)
        ).astype(np.float32).astype(bf16)
        in_maps.append({
            "x_fm": np.ascontiguousarray(seqs[b_idx, q0:q0 + T].T),
            "maskd": maskd,
            "cosf": np.ascontiguousarray(np.concatenate([cos_t, cos_t], 0)),
            "sinf": np.ascontiguousarray(np.concatenate([-sin_t, sin_t], 0)),
            "pswap": pswap, "onesf": np.ones((128, 128), np.float32),
            "wstack": wstack, "bstack": bstack, "lng": lng, "lnb": lnb,
        })
    return in_maps


def _get_program(unroll=1):
    key = ("nc", unroll)
    if key not in _CACHE:
        os.environ.setdefault("JAX_PLATFORMS", "")
        _CACHE[key] = _build_program(unroll=unroll)
    return _CACHE[key]


class _Runner:
    """Compile-once jitted SPMD runner over the axon/PJRT path."""

    def __init__(self, nc):
        import jax
        from jax.experimental.shard_map import shard_map
        from jax.sharding import Mesh, PartitionSpec, NamedSharding
        import concourse.bass2jax as bass2jax
        import concourse.mybir as mybir

        self.jax = jax
        self.nc = nc
        bass2jax.install_neuronx_cc_hook()
        partition_name = (nc.partition_id_tensor.name
                          if nc.partition_id_tensor else None)
        in_names, out_names, out_avals, zero_outs = [], [], [], []
        for alloc in nc.m.functions[0].allocations:
            if not isinstance(alloc, mybir.MemoryLocationSet):
                continue
            name = alloc.memorylocations[0].name
            if alloc.kind == "ExternalInput":
                if name != partition_name:
                    in_names.append(name)
            elif alloc.kind == "ExternalOutput":
                out_names.append(name)
                shape = tuple(alloc.tensor_shape)
                dtype = mybir.dt.np(alloc.dtype)
                out_avals.append(jax.core.ShapedArray(shape, dtype))
                zero_outs.append(np.zeros(shape, dtype))
        self.in_names, self.out_names = in_names, out_names
        self.zero_outs = zero_outs
        n_params = len(in_names)
        all_names = in_names + out_names + (
            [partition_name] if partition_name else [])

        def _body(*args):
            operands = list(args)
            if partition_name is not None:
                operands.append(bass2jax.partition_id_tensor())
            return tuple(bass2jax._bass_exec_p.bind(
                *operands, out_avals=tuple(out_avals),
                in_names=tuple(all_names), out_names=tuple(out_names),
                lowering_input_output_aliases=(),
                sim_require_finite=True, sim_require_nnan=True, nc=nc))

        mesh = Mesh(np.asarray(jax.devices()[:NC]), ("core",))
        n_outs = len(out_names)
        self.fn = jax.jit(
            shard_map(_body, mesh=mesh,
                      in_specs=(PartitionSpec("core"),) * (n_params + n_outs),
                      out_specs=(PartitionSpec("core"),) * n_outs,
                      check_rep=False),
            donate_argnums=tuple(range(n_params, n_params + n_outs)),
            keep_unused=True)
        self.shard = NamedSharding(mesh, PartitionSpec("core"))

    def put_inputs(self, in_maps):
        jax = self.jax
        concat_in = [
            np.concatenate([np.asarray(in_maps[c][nm])[None]
                            for c in range(NC)], axis=0)
            .reshape(NC * in_maps[0][nm].shape[0], *in_maps[0][nm].shape[1:])
            for nm in self.in_names]
        return [jax.device_put(a, self.shard) for a in concat_in]

    def fresh_zeros(self):
        jax = self.jax
        return [jax.device_put(
            np.zeros((NC * z.shape[0], *z.shape[1:]), z.dtype), self.shard)
            for z in self.zero_outs]

    def run(self, in_arrs):
        outs = self.fn(*in_arrs, *self.fresh_zeros())
        self.jax.block_until_ready(outs)
        return outs

    def times(self, in_arrs, iters):
        jax = self.jax
        self.run(in_arrs)  # warmup
        ts = []
        for _ in range(iters):
            zs = self.fresh_zeros()
            jax.block_until_ready(zs)
            t0 = time.perf_counter()
            outs = self.fn(*in_arrs, *zs)
            jax.block_until_ready(outs)
            ts.append(time.perf_counter() - t0)
        return ts


def _out_to_full(runner, outs):
    out = np.zeros((B, L, D), np.float32)
    arr0 = np.asarray(outs[runner.out_names.index("out_fm")]).reshape(NC, D, T)
    for c in range(NC):
        b_idx, q0 = c // 4, (c % 4) * T
        out[b_idx, q0:q0 + T] = arr0[c].T
    return out


def kernel(**inputs):
    from concourse.bass_utils import run_bass_kernel_spmd
    in_maps = _host_prep(inputs)
    nc = _get_program(unroll=1)
    res = run_bass_kernel_spmd(nc, in_maps, core_ids=list(range(NC)))
    out = np.zeros((B, L, D), np.float32)
    for c in range(NC):
        b_idx, q0 = c // 4, (c % 4) * T
        out[b_idx, q0:q0 + T] = res.results[c]["out_fm"].T
    return out


def bench_hw(inputs, unroll=6, iters=8):
    """Correctness output + device-time estimate via unrolled NEFF diff."""
    in_maps = _host_prep(inputs)
    r1 = _CACHE.setdefault("runner1", _Runner(_get_program(unroll=1)))
    rN = _CACHE.setdefault(f"runner{unroll}",
                           _Runner(_get_program(unroll=unroll)))
    in1 = r1.put_inputs(in_maps)
    inN = rN.put_inputs(in_maps)
    outs = r1.run(in1)
    full = _out_to_full(r1, outs)
    # interleave timing rounds so dispatch-floor drift cancels
    r1.run(in1)
    rN.run(inN)
    t1s, tNs = [], []
    for _ in range(iters):
        t1s.extend(r1.times(in1, 1))
        tNs.extend(rN.times(inN, 1))
    t1, tN = min(t1s), min(tNs)
    est = (tN - t1) / (unroll - 1)
    # also check unrolled output matches (reps are idempotent)
    fullN = _out_to_full(rN, rN.run(inN))
    assert np.allclose(full, fullN, atol=1e-5), "unrolled output mismatch"
    return full, est, {"t1": t1, "tN": tN, "unroll": unroll,
                       "t1s": t1s, "tNs": tNs}


# revision 37
# speedup vs baseline: 1.1439x; 1.0395x over previous
"""Trainium2 Bass kernel for nn_BaselineModel_55018531061929 (2-layer HSTU-style
dense transformer, B=2 L=2048 D=1024 H=8, SiLU attention).

Sharding (plan D): token-sharded projections + head-sharded attention via
8-core AllToAll. 8 cores = 2 batches x 4 token blocks of 512. Each core:
  - computes Q/K/U/V (all heads, own 512 tokens) locally from fp32 h,
    weights in bf16 (stationary), rope applied locally, outputs bf16;
  - AllToAll #1 reshard: core c receives head-c Q/K/U/V for all 2048 tokens
    of both batches (blocks 0-3 = batch 0, 4-7 = batch 1);
  - attention for head c on 2 batch instances with a STRUCTURAL causal
    triangular loop (query chunk qc only attends key tiles kt <= 4qc+3,
    diagonal 4 tiles masked from data) - perfectly load balanced;
  - AllToAll #2 returns attention outputs token-sharded; Wo/FFN/LN local
    in fp32 with bf16 stationary weights.
"""

import os
import time

import numpy as np

B, L, D, H, NL = 2, 2048, 1024, 8, 2
HD = D // H
EPS = 1e-8
NC = 8
T = 512            # tokens per core
DT = D // 128      # 8 d-tiles
G8 = [[0, 1, 2, 3, 4, 5, 6, 7]]

_CACHE = {}


# --------------------------------------------------------------------------
# device program
# --------------------------------------------------------------------------

def _build_program(sim=False, unroll=1):
    import concourse.bacc as bacc
    import concourse.mybir as mybir
    import concourse.tile as tile
    from concourse.masks import make_identity

    f32 = mybir.dt.float32
    f32r = mybir.dt.float32r
    bf16 = mybir.dt.bfloat16
    AF = mybir.ActivationFunctionType

    nc = bacc.Bacc("TRN2", target_bir_lowering=False, debug=False,
                   num_devices=1 if sim else NC)

    # ---- I/O ----
    x_in = nc.dram_tensor("x_fm", [D, T], f32r, kind="ExternalInput")
    maskd_in = nc.dram_tensor("maskd", [128, 4, T], bf16, kind="ExternalInput")
    cos_in = nc.dram_tensor("cosf", [128, T], f32, kind="ExternalInput")
    sin_in = nc.dram_tensor("sinf", [128, T], f32, kind="ExternalInput")
    psw_in = nc.dram_tensor("pswap", [128, 128], f32r, kind="ExternalInput")
    w_in = nc.dram_tensor("wstack", [7 * NL, 8, 128, DT, 128], bf16,
                          kind="ExternalInput")
    ones_in = nc.dram_tensor("onesf", [128, 128], f32r, kind="ExternalInput")
    b_in = nc.dram_tensor("bstack", [7 * NL, 128, 8], f32, kind="ExternalInput")
    lng_in = nc.dram_tensor("lng", [2 * NL + 1, 128, DT], f32, kind="ExternalInput")
    lnb_in = nc.dram_tensor("lnb", [2 * NL + 1, 128, DT], f32, kind="ExternalInput")
    out_t = nc.dram_tensor("out_fm", [D, T], f32r, kind="ExternalOutput")

    W_Q, W_K, W_U, W_V, W_O, W_1, W_2 = range(7)
    INV_SQRT_HD = float(1.0 / np.sqrt(HD))

    with tile.TileContext(nc) as tc:
        with (
            tc.tile_pool(name="const", bufs=1) as constp,
            tc.tile_pool(name="acts", bufs=1) as acts,
            tc.tile_pool(name="wcol", bufs=10) as wcolp,
            tc.tile_pool(name="tmp", bufs=5) as tmpp,
            tc.tile_pool(name="small", bufs=4) as smallp,
            tc.tile_pool(name="krp", bufs=3) as krp,
            tc.tile_pool(name="att", bufs=1) as attp,
            tc.tile_pool(name="vtp", bufs=2) as vtp,
            tc.tile_pool(name="wtsp", bufs=18) as wtsp,
            tc.tile_pool(name="bcp", bufs=2) as bcp,
            tc.tile_pool(name="psc", bufs=3, space="PSUM") as pscp,
            tc.tile_pool(name="shr", bufs=2, space="PSUM") as shrp,
            tc.tile_pool(name="dram", bufs=1, space="DRAM") as dramp,
        ):
            # ---- constants ----
            ones_sb = constp.tile([128, 128], f32r, name="ones_sb")
            nc.sync.dma_start(ones_sb[:], ones_in[:])
            ones_col = ones_sb[:, 0:1]
            ones_f = constp.tile([1, 128], f32, name="ones_f")
            nc.vector.memset(ones_f[:], 1.0)
            ones_row = ones_f[0:1, :]
            eps_col = constp.tile([128, 1], f32, name="eps_col")
            nc.vector.memset(eps_col[:], EPS)
            x_sb = constp.tile([128, DT, T], f32r, name="x_sb")
            x_in_t = x_in.ap().rearrange("(dt p) t -> p dt t", p=128)
            for dt in range(DT):
                nc.sync.dma_start(x_sb[:, dt, :], x_in_t[:, dt, :])
            maskd_sb = constp.tile([128, 4, T], bf16, name="maskd_sb")
            nc.sync.dma_start(maskd_sb[:], maskd_in.ap())
            cos_sb = constp.tile([128, T], f32, name="cos_sb")
            nc.sync.dma_start(cos_sb[:], cos_in[:])
            sin_sb = constp.tile([128, T], f32, name="sin_sb")
            nc.sync.dma_start(sin_sb[:], sin_in[:])
            psw_sb = constp.tile([128, 128], f32r, name="psw_sb")
            nc.sync.dma_start(psw_sb[:], psw_in[:])
            bcol_sb = constp.tile([128, 7 * NL, 8], f32, name="bcol_sb")
            nc.sync.dma_start(bcol_sb[:], b_in.ap().rearrange("w p c -> p w c"))
            lng_sb = constp.tile([128, 2 * NL + 1, DT], f32, name="lng_sb")
            nc.sync.dma_start(lng_sb[:], lng_in.ap().rearrange("w p c -> p w c"))
            lnb_sb = constp.tile([128, 2 * NL + 1, DT], f32, name="lnb_sb")
            nc.sync.dma_start(lnb_sb[:], lnb_in.ap().rearrange("w p c -> p w c"))
            identb = constp.tile([128, 128], bf16, name="identb")
            make_identity(nc, identb)

            # ---- collective buffers ----
            a2a1a_in = [dramp.tile([8, 2, 128, T], bf16, name=f"a2a1a_in{l}")
                        for l in range(NL)]
            a2a1a_out = [dramp.tile([8, 2, 128, T], bf16, name=f"a2a1a_out{l}")
                         for l in range(NL)]
            a2a1b_in = [dramp.tile([8, 128, T], bf16, name=f"a2a1b_in{l}")
                        for l in range(NL)]
            a2a1b_out = [dramp.tile([8, 128, T], bf16, name=f"a2a1b_out{l}")
                         for l in range(NL)]
            a2a2_in = [dramp.tile([8, 128, T], bf16, name=f"a2a2_in{l}")
                       for l in range(NL)]
            a2a2_out = [dramp.tile([8, 128, T], bf16, name=f"a2a2_out{l}")
                        for l in range(NL)]
            wu_in = dramp.tile([8, 128, 16], bf16, name="wu_in")
            wu_out = dramp.tile([8, 128, 16], bf16, name="wu_out")

            def a2a(ins, outs):
                if sim:
                    for j in range(8):
                        nc.sync.dma_start(outs[j], ins[j])
                else:
                    nc.gpsimd.collective_compute(
                        "AllToAll", mybir.AluOpType.bypass,
                        replica_groups=G8, ins=[ins[:]], outs=[outs[:]])

            def load_wcol(widx, ot):
                w = wcolp.tile([128, DT, 128], bf16, name="wct", tag="wct")
                nc.sync.dma_start(w[:], w_in[widx, ot])
                return w

            def ln_stats():
                """LN stats over x_sb -> bc[:,0,:]=mean bcast, bc[:,1,:]=istd
                bcast."""
                ps_sum = shrp.tile([1, T], f32, name="ps_sum", tag="shr",
                                   padded_shape=[128, T])
                ps_sq = shrp.tile([1, T], f32, name="ps_sq", tag="shr",
                                  padded_shape=[128, T])
                for dt in range(DT):
                    sqv = tmpp.tile([128, T], f32r, name="sqv", tag="tmp")
                    nc.scalar.square(sqv[:], x_sb[:, dt, :])
                    nc.tensor.matmul(ps_sum[:], ones_col[:], x_sb[:, dt, :],
                                     start=dt == 0, stop=dt == DT - 1)
                    nc.tensor.matmul(ps_sq[:], ones_col[:], sqv[:],
                                     start=dt == 0, stop=dt == DT - 1)
                s_mean = smallp.tile([1, T], f32, name="s_mean", tag="sm")
                nc.vector.tensor_scalar_mul(s_mean[:], ps_sum[:], 1.0 / D)
                bc = bcp.tile([128, 2, T], f32, name="bc", tag="bc")
                bm_ps = shrp.tile([128, T], f32, name="bm_ps", tag="shr")
                nc.tensor.matmul(bm_ps[:], ones_row[:], s_mean[:],
                                 start=True, stop=True)
                nc.vector.tensor_copy(bc[:, 0, :], bm_ps[:])
                s_var = smallp.tile([1, T], f32, name="s_var", tag="sm")
                nc.vector.tensor_scalar_mul(s_var[:], ps_sq[:], 1.0 / D)
                s_msq = smallp.tile([1, T], f32, name="s_msq", tag="sm")
                nc.vector.tensor_mul(s_msq[:], s_mean[:], s_mean[:])
                nc.vector.tensor_sub(s_var[:], s_var[:], s_msq[:])
                s_std = smallp.tile([1, T], f32, name="s_std", tag="sm")
                nc.scalar.activation(s_std[:], s_var[:], AF.Sqrt, bias=eps_col[:1])
                s_istd = smallp.tile([1, T], f32, name="s_istd", tag="sm")
                nc.vector.reciprocal_approx_fast(s_istd[:], s_std[:])
                bi_ps = shrp.tile([128, T], f32, name="bi_ps", tag="shr")
                nc.tensor.matmul(bi_ps[:], ones_row[:], s_istd[:],
                                 start=True, stop=True)
                nc.vector.tensor_copy(bc[:, 1, :], bi_ps[:])
                return bc

            def ln_norm():
                """xb = bf16((x - mean)*istd); gamma/beta live in the folded
                weights so projections consume xb with a plain bias ACT."""
                bc = ln_stats()
                xb = acts.tile([128, DT, T], bf16, name="xb", tag="bigA")
                for dt in range(DT):
                    t1 = tmpp.tile([128, T], f32, name="t1", tag="tmp")
                    nc.vector.tensor_sub(t1[:], x_sb[:, dt, :], bc[:, 0, :])
                    nc.vector.tensor_mul(xb[:, dt, :], t1[:], bc[:, 1, :])
                return xb

            def layernorm_full(idx):
                """Classic layernorm of x_sb (final LN only), f32r out."""
                bc = ln_stats()
                h = acts.tile([128, DT, T], f32r, name="hf", tag="bigF")
                for dt in range(DT):
                    t1 = tmpp.tile([128, T], f32, name="t1", tag="tmp")
                    nc.vector.tensor_sub(t1[:], x_sb[:, dt, :], bc[:, 0, :])
                    nc.vector.tensor_mul(t1[:], t1[:], bc[:, 1, :])
                    nc.scalar.activation(h[:, dt, :], t1[:], AF.Identity,
                                         bias=lnb_sb[:, idx, dt:dt + 1],
                                         scale=lng_sb[:, idx, dt:dt + 1])
                return h

            def proj_pair_psum(widx, otp, rhs_tile):
                """[128, 2, T] psum: halves = ot 2*otp, 2*otp+1 accumulation."""
                w0 = load_wcol(widx, 2 * otp)
                w1 = load_wcol(widx, 2 * otp + 1)
                ps = pscp.tile([128, 2, T], f32, name="ps_p", tag="psc")
                for dt in range(DT):
                    nc.tensor.matmul(ps[:, 0, :], w0[:, dt, :],
                                     rhs_tile[:, dt, :],
                                     start=dt == 0, stop=dt == DT - 1)
                    nc.tensor.matmul(ps[:, 1, :], w1[:, dt, :],
                                     rhs_tile[:, dt, :],
                                     start=dt == 0, stop=dt == DT - 1)
                return ps

            def rope_to_bf16(dst_ap, src_tile):
                """dst(bf16) = src*cosf + (pswap@src)*sinf, one rounding."""
                psw = shrp.tile([128, T], f32, name="psw_ps", tag="shr")
                nc.tensor.matmul(psw[:], psw_sb[:], src_tile[:],
                                 start=True, stop=True)
                t1 = tmpp.tile([128, T], f32, name="rt1", tag="tmp")
                nc.vector.tensor_mul(t1[:], src_tile[:], cos_sb[:])
                t2 = tmpp.tile([128, T], f32, name="rt2", tag="tmp")
                nc.vector.tensor_mul(t2[:], psw[:], sin_sb[:])
                nc.vector.tensor_add(dst_ap, t1[:], t2[:])

            # warmup collective: absorbs first-op slowness off the critical path
            wuc = constp.tile([128, 16], bf16, name="wuc")
            nc.vector.tensor_copy(wuc[:], ones_sb[:, 0:16])
            for j in range(8):
                nc.sync.dma_start(wu_in[j], wuc[:])
            a2a(wu_in, wu_out)

            for rep in range(unroll):
                if rep > 0:
                    for dt in range(DT):
                        nc.sync.dma_start(x_sb[:, dt, :], x_in_t[:, dt, :])
                for layer in range(NL):
                    wofs = 7 * layer
                    xb = ln_norm()

                    # ---- Q/K projections -> A2A1a; V -> A2A1b; U local ----
                    for m, widx in ((0, W_Q), (1, W_K), (3, W_V)):
                        for otp in range(H // 2):
                            ps = proj_pair_psum(wofs + widx, otp, xb)
                            for j in range(2):
                                ot = 2 * otp + j
                                kr = krp.tile([128, T], bf16, name="kr",
                                              tag="kr")
                                if m < 2:  # Q, K: bias then rope
                                    qt = tmpp.tile([128, T], f32r, name="qt",
                                                   tag="tmp")
                                    nc.scalar.activation(
                                        qt[:], ps[:, j, :], AF.Identity,
                                        bias=bcol_sb[:, wofs + widx, ot:ot + 1])
                                    rope_to_bf16(kr[:], qt)
                                    nc.sync.dma_start(
                                        a2a1a_in[layer][ot, m], kr[:])
                                else:  # V
                                    nc.scalar.activation(
                                        kr[:], ps[:, j, :], AF.Identity,
                                        bias=bcol_sb[:, wofs + widx, ot:ot + 1])
                                    nc.sync.dma_start(
                                        a2a1b_in[layer][ot], kr[:])
                        if m == 1:
                            a2a(a2a1a_in[layer], a2a1a_out[layer])
                        elif m == 3:
                            a2a(a2a1b_in[layer], a2a1b_out[layer])
                    # U projection stays local (fills the A2A flight time)
                    u_sb = acts.tile([128, H, T], bf16, name="u_sb", tag="u")
                    for otp in range(H // 2):
                        ps = proj_pair_psum(wofs + W_U, otp, xb)
                        for j in range(2):
                            ot = 2 * otp + j
                            nc.scalar.activation(
                                u_sb[:, ot, :], ps[:, j, :], AF.Identity,
                                bias=bcol_sb[:, wofs + W_U, ot:ot + 1])

                    # ---- attention: head `core`, 2 batch instances woven so
                    # TensorE (scores+AV) and ACT (silu) stay concurrently
                    # busy: big-qc scores first, then scores/AV 1:1 ----
                    qf, kf, vf, kfl, vt, wts, pav = {}, {}, {}, {}, {}, {}, {}
                    for i in range(2):
                        base = 4 * i
                        qf[i] = attp.tile([128, 4, T], bf16, name="qf",
                                          tag=f"qf{i}")
                        kf[i] = attp.tile([128, 4, T], bf16, name="kf",
                                          tag=f"kf{i}")
                        vf[i] = attp.tile([128, 4, T], bf16, name="vf",
                                          tag=f"vf{i}")
                        for blk in range(4):
                            nc.sync.dma_start(qf[i][:, blk, :],
                                              a2a1a_out[layer][base + blk, 0])
                            nc.sync.dma_start(kf[i][:, blk, :],
                                              a2a1a_out[layer][base + blk, 1])
                            nc.sync.dma_start(vf[i][:, blk, :],
                                              a2a1b_out[layer][base + blk])
                        kfl[i] = kf[i][:].rearrange("p b t -> p (b t)")

                    def s_pair(i, qc, ktp):
                        psc = pscp.tile([128, 2, T], f32, name="psc",
                                        tag="psc")
                        for j in range(2):
                            kt = 2 * ktp + j
                            nc.tensor.matmul(
                                psc[:, j, :],
                                kfl[i][:, kt * 128:(kt + 1) * 128],
                                qf[i][:, qc, :], start=True, stop=True)
                        wt = wtsp.tile([128, 2, T], bf16, name="wt", tag="wt")
                        nc.scalar.activation(wt[:], psc[:], AF.Silu,
                                             scale=INV_SQRT_HD)
                        if ktp >= 2 * qc:  # diagonal pair: mask
                            dj = 2 * (ktp - 2 * qc)
                            nc.vector.tensor_mul(wt[:], wt[:],
                                                 maskd_sb[:, dj:dj + 2, :])
                        wts[(i, qc, ktp)] = wt

                    def t_set(i):
                        vt[i] = vtp.tile([128, 16, 128], bf16, name="vt",
                                         tag="vt")
                        vflat = vf[i][:].rearrange("p b t -> p (b t)")
                        for kt in range(16):
                            pst = shrp.tile([128, 128], bf16, name="pst",
                                            tag="shr", padded_shape=[128, 512])
                            nc.tensor.transpose(
                                pst[:], vflat[:, kt * 128:(kt + 1) * 128],
                                identb[:])
                            nc.vector.tensor_copy(vt[i][:, kt, :], pst[:])

                    def av_pair(i, qc, ktp):
                        npair = 2 * qc + 2
                        if ktp == 0:
                            pav[(i, qc)] = shrp.tile([128, T], f32,
                                                     name="pav", tag="shr")
                        wt = wts.pop((i, qc, ktp))
                        for j in range(2):
                            kt = 2 * ktp + j
                            nc.tensor.matmul(
                                pav[(i, qc)][:], vt[i][:, kt, :], wt[:, j, :],
                                start=kt == 0, stop=kt == 2 * npair - 1)
                        if ktp == npair - 1:
                            ao = krp.tile([128, T], bf16, name="ao", tag="kr")
                            nc.scalar.activation(ao[:], pav[(i, qc)][:],
                                                 AF.Identity)
                            nc.sync.dma_start(a2a2_in[layer][4 * i + qc],
                                              ao[:])

                    # phase 1: qc3 scores for both instances, then transposes
                    for i in range(2):
                        for ktp in range(8):
                            s_pair(i, 3, ktp)
                    t_set(0)
                    t_set(1)
                    # phase 2: weave remaining scores (qc 2,1,0) with AV pairs
                    s_stream = [(i, qc, ktp) for qc in (2, 1, 0)
                                for i in (0, 1) for ktp in range(2 * qc + 2)]
                    av_stream = [(i, qc, ktp) for qc in (3, 2, 1, 0)
                                 for i in (0, 1) for ktp in range(2 * qc + 2)]
                    si = ai = 0
                    while si < len(s_stream) or ai < len(av_stream):
                        if si < len(s_stream):
                            s_pair(*s_stream[si])
                            si += 1
                        if ai < len(av_stream):
                            av_pair(*av_stream[ai])
                            ai += 1
                    a2a(a2a2_in[layer], a2a2_out[layer])

                    # ---- U gating + output projection + residual ----
                    aa = attp.tile([128, 8, T], bf16, name="aa", tag="aa")
                    for s in range(8):
                        nc.sync.dma_start(aa[:, s, :], a2a2_out[layer][s])
                    au = attp.tile([128, 8, T], bf16, name="au", tag="au")
                    for s in range(8):
                        nc.vector.tensor_mul(au[:, s, :], aa[:, s, :],
                                             u_sb[:, s, :])
                    for otp in range(DT // 2):
                        ps = proj_pair_psum(wofs + W_O, otp, au)
                        for j in range(2):
                            ot = 2 * otp + j
                            otmp = tmpp.tile([128, T], f32, name="otmp",
                                             tag="tmp")
                            nc.vector.tensor_scalar_add(
                                otmp[:], ps[:, j, :],
                                bcol_sb[:, wofs + W_O, ot:ot + 1])
                            nc.vector.tensor_add(x_sb[:, ot, :],
                                                 x_sb[:, ot, :], otmp[:])

                    # ---- FFN (LN2 gain/shift folded into W1) ----
                    xb2 = ln_norm()
                    p_sb = acts.tile([128, DT, T], f32, name="p_sb", tag="p")
                    for otp in range(DT // 2):
                        ps = proj_pair_psum(wofs + W_1, otp, xb2)
                        for j in range(2):
                            ot = 2 * otp + j
                            nc.scalar.activation(
                                p_sb[:, ot, :], ps[:, j, :], AF.Identity,
                                bias=bcol_sb[:, wofs + W_1, ot:ot + 1])
                    gp = acts.tile([128, DT, T], bf16, name="gp", tag="bigA")
                    for ot in range(DT):
                        sp = tmpp.tile([128, T], f32, name="sp", tag="tmp")
                        nc.scalar.activation(sp[:], p_sb[:, ot, :], AF.Silu)
                        nc.vector.tensor_mul(gp[:, ot, :], p_sb[:, ot, :], sp[:])
                    for otp in range(DT // 2):
                        ps = proj_pair_psum(wofs + W_2, otp, gp)
                        for j in range(2):
                            ot = 2 * otp + j
                            ftmp = tmpp.tile([128, T], f32, name="ftmp",
                                             tag="tmp")
                            nc.vector.tensor_scalar_add(
                                ftmp[:], ps[:, j, :],
                                bcol_sb[:, wofs + W_2, ot:ot + 1])
                            nc.vector.tensor_add(x_sb[:, ot, :],
                                                 x_sb[:, ot, :], ftmp[:])

                # ---- final layernorm + output ----
                hf = layernorm_full(2 * NL)
                out_t_t = out_t.ap().rearrange("(dt p) t -> p dt t", p=128)
                for dt in range(DT):
                    nc.sync.dma_start(out_t_t[:, dt, :], hf[:, dt, :])

    nc.compile()
    return nc


# --------------------------------------------------------------------------
# host-side preparation
# --------------------------------------------------------------------------

def _host_prep(inputs):
    import ml_dtypes
    bf16 = ml_dtypes.bfloat16

    seqs = np.asarray(inputs["seqs"], np.float32)
    mask = np.asarray(inputs["attn_mask"])

    perm128 = np.concatenate([np.arange(0, 128, 2), np.arange(1, 128, 2)])
    perm_full = np.concatenate([h * 128 + perm128 for h in range(H)])

    def wprep(W):
        WT = np.ascontiguousarray(W.T)
        return np.ascontiguousarray(
            WT.reshape(DT, 128, 8, 128).transpose(2, 1, 0, 3))

    def bcolv(b):
        return np.ascontiguousarray(b.reshape(8, 128).T)

    def lncol(v):
        return np.ascontiguousarray(v.reshape(DT, 128).T)

    wstack, bstack = [], []
    for i in range(NL):
        ln1g = np.asarray(inputs["ln1_g"][i], np.float32)
        ln1b = np.asarray(inputs["ln1_b"][i], np.float32)
        ln2g = np.asarray(inputs["ln2_g"][i], np.float32)
        ln2b = np.asarray(inputs["ln2_b"][i], np.float32)
        for nm in ["Wq", "Wk", "Wu", "Wv", "Wo", "W1", "W2"]:
            Wm = np.asarray(inputs[nm][i], np.float32)
            bm = np.asarray(inputs["b" + nm[1:].lower()][i], np.float32)
            # fold the feeding layernorm's gain/shift into W and b:
            # W @ (g*xn + s) + b = (W*g) @ xn + (b + W @ s)
            if nm in ("Wq", "Wk", "Wu", "Wv"):
                bm = bm + Wm @ ln1b
                Wm = Wm * ln1g[None, :]
            elif nm == "W1":
                bm = bm + Wm @ ln2b
                Wm = Wm * ln2g[None, :]
            if nm in ("Wq", "Wk"):
                Wm = Wm[perm_full]
                bm = bm[perm_full]
            wstack.append(wprep(Wm))
            bstack.append(bcolv(bm))
    wstack = np.ascontiguousarray(np.stack(wstack)).astype(bf16)
    bstack = np.ascontiguousarray(np.stack(bstack), dtype=np.float32)

    lng = np.stack([lncol(np.asarray(inputs["ln1_g"][0], np.float32)),
                    lncol(np.asarray(inputs["ln2_g"][0], np.float32)),
                    lncol(np.asarray(inputs["ln1_g"][1], np.float32)),
                    lncol(np.asarray(inputs["ln2_g"][1], np.float32)),
                    lncol(np.asarray(inputs["lnf_g"], np.float32))])
    lnb = np.stack([lncol(np.asarray(inputs["ln1_b"][0], np.float32)),
                    lncol(np.asarray(inputs["ln2_b"][0], np.float32)),
                    lncol(np.asarray(inputs["ln1_b"][1], np.float32)),
                    lncol(np.asarray(inputs["ln2_b"][1], np.float32)),
                    lncol(np.asarray(inputs["lnf_b"], np.float32))])
    lng = np.ascontiguousarray(lng, dtype=np.float32)
    lnb = np.ascontiguousarray(lnb, dtype=np.float32)

    pos = np.arange(L, dtype=np.float32)
    ar = np.arange(0, HD, 2).astype(np.float32) / np.float32(HD)
    freqs = np.float32(1.0) / np.power(np.float32(10000.0), ar, dtype=np.float32)
    ang = pos[:, None] * freqs[None, :]
    sin_full, cos_full = np.sin(ang).astype(np.float32), np.cos(ang).astype(np.float32)

    pswap = np.zeros((128, 128), np.float32)
    for i in range(64):
        pswap[i, 64 + i] = 1.0
        pswap[64 + i, i] = 1.0

    in_maps = []
    for c in range(NC):
        b_idx, q0 = c // 4, (c % 4) * T
        cos_t = cos_full[q0:q0 + T].T
        sin_t = sin_full[q0:q0 + T].T
        # diagonal-relative causal mask: maskd[k, j, q] = allow(query q,
        # key 128j+k) within a 512x512 diagonal block
        mdiag = mask[b_idx, 0:T, 0:T]          # [q, k] bool (tril)
        maskd = np.ascontiguousarray(
            mdiag.T.reshape(4, 128, T).transpose(1, 0, 2)
        ).astype(np.float32).astype(bf16)
        in_maps.append({
            "x_fm": np.ascontiguousarray(seqs[b_idx, q0:q0 + T].T),
            "maskd": maskd,
            "cosf": np.ascontiguousarray(np.concatenate([cos_t, cos_t], 0)),
            "sinf": np.ascontiguousarray(np.concatenate([-sin_t, sin_t], 0)),
            "pswap": pswap, "onesf": np.ones((128, 128), np.float32),
            "wstack": wstack, "bstack": bstack, "lng": lng, "lnb": lnb,
        })
    return in_maps


def _get_program(unroll=1):
    key = ("nc", unroll)
    if key not in _CACHE:
        os.environ.setdefault("JAX_PLATFORMS", "")
        _CACHE[key] = _build_program(unroll=unroll)
    return _CACHE[key]


class _Runner:
    """Compile-once jitted SPMD runner over the axon/PJRT path."""

    def __init__(self, nc):
        import jax
        from jax.experimental.shard_map import shard_map
        from jax.sharding import Mesh, PartitionSpec, NamedSharding
        import concourse.bass2jax as bass2jax
        import concourse.mybir as mybir

        self.jax = jax
        self.nc = nc
        bass2jax.install_neuronx_cc_hook()
        partition_name = (nc.partition_id_tensor.name
                          if nc.partition_id_tensor else None)
        in_names, out_names, out_avals, zero_outs = [], [], [], []
        for alloc in nc.m.functions[0].allocations:
            if not isinstance(alloc, mybir.MemoryLocationSet):
                continue
            name = alloc.memorylocations[0].name
            if alloc.kind == "ExternalInput":
                if name != partition_name:
                    in_names.append(name)
            elif alloc.kind == "ExternalOutput":
                out_names.append(name)
                shape = tuple(alloc.tensor_shape)
                dtype = mybir.dt.np(alloc.dtype)
                out_avals.append(jax.core.ShapedArray(shape, dtype))
                zero_outs.append(np.zeros(shape, dtype))
        self.in_names, self.out_names = in_names, out_names
        self.zero_outs = zero_outs
        n_params = len(in_names)
        all_names = in_names + out_names + (
            [partition_name] if partition_name else [])

        def _body(*args):
            operands = list(args)
            if partition_name is not None:
                operands.append(bass2jax.partition_id_tensor())
            return tuple(bass2jax._bass_exec_p.bind(
                *operands, out_avals=tuple(out_avals),
                in_names=tuple(all_names), out_names=tuple(out_names),
                lowering_input_output_aliases=(),
                sim_require_finite=True, sim_require_nnan=True, nc=nc))

        mesh = Mesh(np.asarray(jax.devices()[:NC]), ("core",))
        n_outs = len(out_names)
        self.fn = jax.jit(
            shard_map(_body, mesh=mesh,
                      in_specs=(PartitionSpec("core"),) * (n_params + n_outs),
                      out_specs=(PartitionSpec("core"),) * n_outs,
                      check_rep=False),
            donate_argnums=tuple(range(n_params, n_params + n_outs)),
            keep_unused=True)
        self.shard = NamedSharding(mesh, PartitionSpec("core"))

    def put_inputs(self, in_maps):
        jax = self.jax
        concat_in = [
            np.concatenate([np.asarray(in_maps[c][nm])[None]
                            for c in range(NC)], axis=0)
            .reshape(NC * in_maps[0][nm].shape[0], *in_maps[0][nm].shape[1:])
            for nm in self.in_names]
        return [jax.device_put(a, self.shard) for a in concat_in]

    def fresh_zeros(self):
        jax = self.jax
        return [jax.device_put(
            np.zeros((NC * z.shape[0], *z.shape[1:]), z.dtype), self.shard)
            for z in self.zero_outs]

    def run(self, in_arrs):
        outs = self.fn(*in_arrs, *self.fresh_zeros())
        self.jax.block_until_ready(outs)
        return outs

    def times(self, in_arrs, iters):
        jax = self.jax
        self.run(in_arrs)  # warmup
        ts = []
        for _ in range(iters):
            zs = self.fresh_zeros()
            jax.block_until_ready(zs)
            t0 = time.perf_counter()
            outs = self.fn(*in_arrs, *zs)
            jax.block_until_ready(outs)
            ts.append(time.perf_counter() - t0)
        return ts


def _out_to_full(runner, outs):
    out = np.zeros((B, L, D), np.float32)
    arr0 = np.asarray(outs[runner.out_names.index("out_fm")]).reshape(NC, D, T)
    for c in range(NC):
        b_idx, q0 = c // 4, (c % 4) * T
        out[b_idx, q0:q0 + T] = arr0[c].T
    return out


def kernel(**inputs):
    from concourse.bass_utils import run_bass_kernel_spmd
    in_maps = _host_prep(inputs)
    nc = _get_program(unroll=1)
    res = run_bass_kernel_spmd(nc, in_maps, core_ids=list(range(NC)))
    out = np.zeros((B, L, D), np.float32)
    for c in range(NC):
        b_idx, q0 = c // 4, (c % 4) * T
        out[b_idx, q0:q0 + T] = res.results[c]["out_fm"].T
    return out


def bench_hw(inputs, unroll=6, iters=8):
    """Correctness output + device-time estimate via unrolled NEFF diff."""
    in_maps = _host_prep(inputs)
    r1 = _CACHE.setdefault("runner1", _Runner(_get_program(unroll=1)))
    rN = _CACHE.setdefault(f"runner{unroll}",
                           _Runner(_get_program(unroll=unroll)))
    in1 = r1.put_inputs(in_maps)
    inN = rN.put_inputs(in_maps)
    outs = r1.run(in1)
    full = _out_to_full(r1, outs)
    # interleave timing rounds so dispatch-floor drift cancels
    r1.run(in1)
    rN.run(inN)
    t1s, tNs = [], []
    for _ in range(iters):
        t1s.extend(r1.times(in1, 1))
        tNs.extend(rN.times(inN, 1))
    t1, tN = min(t1s), min(tNs)
    est = (tN - t1) / (unroll - 1)
    # also check unrolled output matches (reps are idempotent)
    fullN = _out_to_full(rN, rN.run(inN))
    assert np.allclose(full, fullN, atol=1e-5), "unrolled output mismatch"
    return full, est, {"t1": t1, "tN": tN, "unroll": unroll,
                       "t1s": t1s, "tNs": tNs}
